# revision 50
# baseline (speedup 1.0000x reference)
"""Trainium2 Bass kernel for nn_AttentionMemoryBank.

Math (forward; mask/stop_gradient is identity in forward):
    xf     = x.reshape(B, K, N)                 # B=8, K=256, N=4096
    logits = einsum('km,bkn->bmn', mem[0], xf) / sqrt(K)   # (B, M=1024, N)
    attn   = softmax(logits, axis=1)            # over M
    out    = einsum('km,bmn->bkn', mem[0], attn).reshape(B, K, 64, 64)
    return (out, logits)

Distribution: data-parallel over batch — one batch element per NeuronCore
(8 cores), memory bank replicated.

Per-core kernel (bf16 matmuls, f32 PSUM accumulation; exp without
max-subtraction — logits ~ N(0,1) so exp is numerically safe). The N axis
is processed in 1024-wide blocks; per block:
  stage 1 (per 128-row mt tile): L (128m x 1024) = memKM.T @ x   (PE)
           lsb  = L * 1/16  bf16                                 (DVE) -> HBM
           expl = exp(lsb)  bf16                                 (ACT)
  stage 2 (per 128-wide n-tile, lagged half a block so exps settle):
           O (128n x 257) = sum_mt expl_mt[:, nt].T @ [memT | 1] (PE)
             -> O[:, :256] = unnormalized out.T, O[:, 256] = softmax denom
           outT = O[:, :256] * reciprocal(O[:, 256])  bf16       (DVE/ACT) -> HBM
Outputs are stored bf16 on device and upcast on host; the host also
transposes per-core outT (4096,256) -> (256,64,64) while stacking.
"""

import sys

if "/opt/trn_rl_repo" not in sys.path:
    sys.path.insert(0, "/opt/trn_rl_repo")

import numpy as np

B = 8
K = 256
HH = 64
WW = 64
N = HH * WW          # 4096
M = 1024             # mem bank size
NCORES = 8
NCHUNK = 512         # n-columns per stage-1 matmul (PSUM bank = 512 f32)
SCALE = 1.0 / 16.0   # 1/sqrt(K)

KT = K // 128        # 2 contraction tiles for stage 1
MT = M // 128        # 8 logits partition tiles / stage-2 contraction tiles
NCH = N // NCHUNK    # stage-1 chunks per row
NT = N // 128        # stage-2 n-tiles
OB = 4               # outT n-tiles batched per DMA

_cache = {}


def _build():
    from concourse import bacc
    import concourse.tile as tile
    from concourse import mybir
    from contextlib import ExitStack

    f32 = mybir.dt.float32
    bf16 = mybir.dt.bfloat16

    nc = bacc.Bacc("TRN2", target_bir_lowering=False, debug=False,
                   num_devices=NCORES)

    x_d = nc.dram_tensor("x", [K, N], bf16, kind="ExternalInput").ap()
    mkm_d = nc.dram_tensor("mem_km", [K, M], bf16, kind="ExternalInput").ap()
    mt1_d = nc.dram_tensor("mem_t1", [M, K + 1], bf16, kind="ExternalInput").ap()
    # outputs are stored bf16 on device (halves write traffic; the host
    # upcasts) — quantization adds ~1e-3 rel err, small next to the bf16
    # matmul error and far inside tolerance
    lg_d = nc.dram_tensor("logits", [M, N], bf16, kind="ExternalOutput").ap()
    ot_d = nc.dram_tensor("outT", [N, K], bf16, kind="ExternalOutput").ap()

    with tile.TileContext(nc) as tc, ExitStack() as ctx:
        const = ctx.enter_context(tc.tile_pool(name="const", bufs=1))

        x_sb = const.tile([128, KT, N], bf16)
        mkm_sb = const.tile([128, KT, M], bf16)
        mt1_sb = const.tile([128, MT, K + 1], bf16)
        x_re = x_d.rearrange("(t p) n -> p t n", p=128)
        mkm_re = mkm_d.rearrange("(t p) m -> p t m", p=128)
        mt1_re = mt1_d.rearrange("(t p) c -> p t c", p=128)
        # HWDGE issue order is what the first matmuls wait on: tiny first
        # slice of the memory bank, first x block, rest of the bank, the
        # bulk of x, with mt1 landing before stage 2 needs it
        nc.sync.dma_start(out=mkm_sb[:, :, 0:128], in_=mkm_re[:, :, 0:128])
        nc.sync.dma_start(out=x_sb[:, :, 0:512], in_=x_re[:, :, 0:512])
        nc.sync.dma_start(out=mt1_sb[:, 0:2, :], in_=mt1_re[:, 0:2, :])
        nc.sync.dma_start(out=mkm_sb[:, :, 128:M], in_=mkm_re[:, :, 128:M])
        nc.sync.dma_start(out=x_sb[:, :, 512:1024], in_=x_re[:, :, 512:1024])
        nc.sync.dma_start(out=mt1_sb[:, 2:MT, :], in_=mt1_re[:, 2:MT, :])
        nc.sync.dma_start(out=x_sb[:, :, 1024:2048], in_=x_re[:, :, 1024:2048])
        nc.sync.dma_start(out=x_sb[:, :, 2048:3072], in_=x_re[:, :, 2048:3072])
        nc.sync.dma_start(out=x_sb[:, :, 3072:N], in_=x_re[:, :, 3072:N])

        # warm up the PE (HAM clock gate) with throwaway matmuls on zeros
        # while the inputs stream in — PE is idle here anyway
        warm_pool = ctx.enter_context(tc.tile_pool(name="warm", bufs=1))
        warm = warm_pool.tile([128, 512], bf16)
        nc.vector.memset(warm, 0.0)

        psumL = ctx.enter_context(tc.tile_pool(name="psumL", bufs=3, space="PSUM"))
        psumO = ctx.enter_context(tc.tile_pool(name="psumO", bufs=2, space="PSUM"))
        lpool = ctx.enter_context(tc.tile_pool(name="lsb", bufs=12))
        epool = ctx.enter_context(tc.tile_pool(name="expl", bufs=1))
        opool = ctx.enter_context(tc.tile_pool(name="osb", bufs=2))
        rpool = ctx.enter_context(tc.tile_pool(name="rcp", bufs=4))

        # full exp(logits) stays resident: 8 x (128, 4096) bf16 = 64KB/partition
        expl = epool.tile([128, MT, N], bf16)
        ot_re = ot_d.rearrange("(g p) k -> p g k", p=128)

        LW = 2 * NCHUNK           # PSUM L spans 2 banks -> halves DVE/ACT ops
        NB = N // LW              # interleave blocks

        def stage1(base, w, mid=None):
            for mt in range(MT):
                if mt == 6 and mid is not None:
                    mid()
                ms = slice(mt * 128, (mt + 1) * 128)
                L = psumL.tile([128, w], f32, tag="L")
                # kt-major so consecutive matmuls reuse the stationary weights
                for kt in range(KT):
                    for sub in range(w // NCHUNK):
                        ns = slice(base + sub * NCHUNK, base + (sub + 1) * NCHUNK)
                        ps = slice(sub * NCHUNK, (sub + 1) * NCHUNK)
                        nc.tensor.matmul(L[:, ps], mkm_sb[:, kt, ms],
                                         x_sb[:, kt, ns],
                                         start=(kt == 0), stop=(kt == KT - 1))
                lsb = lpool.tile([128, w], bf16, tag="lsb")
                nc.vector.tensor_scalar_mul(lsb, L, SCALE)
                # exp reads the SBUF logits copy so the PSUM slot frees after
                # a single reader (DVE), keeping the PE accumulator pool fluid
                nc.scalar.activation(expl[:, mt, base:base + w], lsb,
                                     mybir.ActivationFunctionType.Exp)
                nc.sync.dma_start(out=lg_d[ms, base:base + w], in_=lsb)

        def stage2(nt0, ntiles, obs):
            nt = nt0
            for ob in obs:
                osb = opool.tile([128, ob, K], bf16, tag="osb")
                for t in range(ob):
                    ts_ = slice(nt * 128, (nt + 1) * 128)
                    O = psumO.tile([128, K + 1], f32, tag="O")
                    for mt in range(MT):
                        nc.tensor.matmul(O, expl[:, mt, ts_], mt1_sb[:, mt, :],
                                         start=(mt == 0), stop=(mt == MT - 1))
                    r = rpool.tile([128, 1], f32, tag="rcp")
                    nc.vector.reciprocal(r, O[:, K:K + 1])
                    # alternate the normalize between ACT and DVE to balance
                    if t % 2 == 0:
                        nc.scalar.mul(osb[:, t, :], O[:, 0:K], r)
                    else:
                        nc.vector.tensor_scalar_mul(osb[:, t, :], O[:, 0:K], r)
                    nt += 1
                nc.sync.dma_start(out=ot_re[:, nt - ob:nt, :], in_=osb)

        # ~2.5us of dummy matmuls so the PE clock gate is warm (and the
        # pipeline primed) by the time the first x bytes land
        Lw = psumL.tile([128, 512], f32, tag="L")
        for _ in range(22):
            nc.tensor.matmul(Lw, warm[:, 0:128], warm, start=True, stop=True)

        # stage 2 of block cc is emitted mid-way through stage 1 of block
        # cc+1, by which point all of block cc's exps have settled
        ntiles = LW // 128

        def make_mid(cc):
            obs = [8] if cc < NB - 1 else [4, 2, 2]
            return lambda: stage2(cc * ntiles, ntiles, obs)

        # first block in 512-wide granules to track the x stream
        stage1(0, NCHUNK)
        stage1(NCHUNK, NCHUNK)
        for cc in range(1, NB):
            stage1(cc * LW, LW, mid=make_mid(cc - 1))
        make_mid(NB - 1)()

    nc.compile()
    return nc


def _get_nc():
    if "nc" not in _cache:
        _cache["nc"] = _build()
    return _cache["nc"]


def _run(x, mem, trace=False, **kwargs):
    import ml_dtypes
    from concourse.bass_utils import run_bass_kernel_spmd

    nc = _get_nc()
    x = np.asarray(x, dtype=np.float32).reshape(B, K, N).astype(ml_dtypes.bfloat16)
    mem0 = np.asarray(mem, dtype=np.float32)[0]                    # (K, M)
    mkm = mem0.astype(ml_dtypes.bfloat16)
    mt1 = np.concatenate(
        [mem0.T, np.ones((M, 1), np.float32)], axis=1
    ).astype(ml_dtypes.bfloat16)                                   # (M, K+1)
    in_maps = [
        {"x": np.ascontiguousarray(x[b]), "mem_km": mkm, "mem_t1": mt1}
        for b in range(B)
    ]
    return run_bass_kernel_spmd(nc, in_maps, core_ids=list(range(NCORES)),
                                trace=trace, **kwargs)


def kernel(x, mask, mem):
    res = _run(x, mem, trace=False)
    logits = np.stack(
        [res.results[b]["logits"].astype(np.float32) for b in range(B)]
    )
    out = np.stack(
        [res.results[b]["outT"].astype(np.float32).T.reshape(K, HH, WW)
         for b in range(B)]
    )
    return out, logits


# revision 51
# speedup vs baseline: 1.0224x; 1.0224x over previous
"""Trainium2 Bass kernel for nn_AttentionMemoryBank.

Math (forward; mask/stop_gradient is identity in forward):
    xf     = x.reshape(B, K, N)                 # B=8, K=256, N=4096
    logits = einsum('km,bkn->bmn', mem[0], xf) / sqrt(K)   # (B, M=1024, N)
    attn   = softmax(logits, axis=1)            # over M
    out    = einsum('km,bmn->bkn', mem[0], attn).reshape(B, K, 64, 64)
    return (out, logits)

Distribution: data-parallel over batch — one batch element per NeuronCore
(8 cores), memory bank replicated.

Per-core kernel (bf16 matmuls, f32 PSUM accumulation; exp without
max-subtraction — logits ~ N(0,1) so exp is numerically safe). The N axis
is processed in 1024-wide blocks; per block:
  stage 1 (per 128-row mt tile): L (128m x 1024) = memKM.T @ x   (PE)
           lsb  = L * 1/16  bf16                                 (DVE) -> HBM
           expl = exp(lsb)  bf16                                 (ACT)
  stage 2 (per 128-wide n-tile, lagged half a block so exps settle):
           O (128n x 257) = sum_mt expl_mt[:, nt].T @ [memT | 1] (PE)
             -> O[:, :256] = unnormalized out.T, O[:, 256] = softmax denom
           outT = O[:, :256] * reciprocal(O[:, 256])  bf16       (DVE/ACT) -> HBM
Outputs are stored bf16 on device and upcast on host; the host also
transposes per-core outT (4096,256) -> (256,64,64) while stacking.
"""

import sys

if "/opt/trn_rl_repo" not in sys.path:
    sys.path.insert(0, "/opt/trn_rl_repo")

import numpy as np

B = 8
K = 256
HH = 64
WW = 64
N = HH * WW          # 4096
M = 1024             # mem bank size
NCORES = 8
NCHUNK = 512         # n-columns per stage-1 matmul (PSUM bank = 512 f32)
SCALE = 1.0 / 16.0   # 1/sqrt(K)

KT = K // 128        # 2 contraction tiles for stage 1
MT = M // 128        # 8 logits partition tiles / stage-2 contraction tiles
NCH = N // NCHUNK    # stage-1 chunks per row
NT = N // 128        # stage-2 n-tiles
OB = 4               # outT n-tiles batched per DMA

_cache = {}


def _build():
    from concourse import bacc
    import concourse.tile as tile
    from concourse import mybir
    from contextlib import ExitStack

    f32 = mybir.dt.float32
    bf16 = mybir.dt.bfloat16

    nc = bacc.Bacc("TRN2", target_bir_lowering=False, debug=False,
                   num_devices=NCORES)

    x_d = nc.dram_tensor("x", [K, N], bf16, kind="ExternalInput").ap()
    mkm_d = nc.dram_tensor("mem_km", [K, M], bf16, kind="ExternalInput").ap()
    mt1_d = nc.dram_tensor("mem_t1", [M, K + 1], bf16, kind="ExternalInput").ap()
    # outputs are stored bf16 on device (halves write traffic; the host
    # upcasts) — quantization adds ~1e-3 rel err, small next to the bf16
    # matmul error and far inside tolerance
    lg_d = nc.dram_tensor("logits", [M, N], bf16, kind="ExternalOutput").ap()
    ot_d = nc.dram_tensor("outT", [N, K], bf16, kind="ExternalOutput").ap()

    with tile.TileContext(nc) as tc, ExitStack() as ctx:
        const = ctx.enter_context(tc.tile_pool(name="const", bufs=1))

        x_sb = const.tile([128, KT, N], bf16)
        mkm_sb = const.tile([128, KT, M], bf16)
        mt1_sb = const.tile([128, MT, K + 1], bf16)
        x_re = x_d.rearrange("(t p) n -> p t n", p=128)
        mkm_re = mkm_d.rearrange("(t p) m -> p t m", p=128)
        mt1_re = mt1_d.rearrange("(t p) c -> p t c", p=128)
        # HWDGE issue order is what the first matmuls wait on: tiny first
        # slice of the memory bank, first x block, rest of the bank, the
        # bulk of x, with mt1 landing before stage 2 needs it
        nc.sync.dma_start(out=mkm_sb[:, :, 0:128], in_=mkm_re[:, :, 0:128])
        nc.sync.dma_start(out=x_sb[:, :, 0:512], in_=x_re[:, :, 0:512])
        nc.sync.dma_start(out=mt1_sb[:, 0:2, :], in_=mt1_re[:, 0:2, :])
        nc.sync.dma_start(out=mkm_sb[:, :, 128:M], in_=mkm_re[:, :, 128:M])
        nc.sync.dma_start(out=x_sb[:, :, 512:1024], in_=x_re[:, :, 512:1024])
        nc.sync.dma_start(out=mt1_sb[:, 2:MT, :], in_=mt1_re[:, 2:MT, :])
        nc.sync.dma_start(out=x_sb[:, :, 1024:2048], in_=x_re[:, :, 1024:2048])
        nc.sync.dma_start(out=x_sb[:, :, 2048:3072], in_=x_re[:, :, 2048:3072])
        nc.sync.dma_start(out=x_sb[:, :, 3072:N], in_=x_re[:, :, 3072:N])

        # warm up the PE (HAM clock gate) with throwaway matmuls on zeros
        # while the inputs stream in — PE is idle here anyway
        warm_pool = ctx.enter_context(tc.tile_pool(name="warm", bufs=1))
        warm = warm_pool.tile([128, 512], bf16)
        nc.vector.memset(warm, 0.0)

        psumL = ctx.enter_context(tc.tile_pool(name="psumL", bufs=3, space="PSUM"))
        psumO = ctx.enter_context(tc.tile_pool(name="psumO", bufs=2, space="PSUM"))
        lpool = ctx.enter_context(tc.tile_pool(name="lsb", bufs=12))
        epool = ctx.enter_context(tc.tile_pool(name="expl", bufs=1))
        opool = ctx.enter_context(tc.tile_pool(name="osb", bufs=2))
        rpool = ctx.enter_context(tc.tile_pool(name="rcp", bufs=4))

        # full exp(logits) stays resident: 8 x (128, 4096) bf16 = 64KB/partition
        expl = epool.tile([128, MT, N], bf16)
        ot_re = ot_d.rearrange("(g p) k -> p g k", p=128)

        LW = 2 * NCHUNK           # PSUM L spans 2 banks -> halves DVE/ACT ops
        NB = N // LW              # interleave blocks

        def stage1(base, w, mid=None):
            for mt in range(MT):
                if mt == 3 and mid is not None:
                    mid()
                ms = slice(mt * 128, (mt + 1) * 128)
                L = psumL.tile([128, w], f32, tag="L")
                # kt-major so consecutive matmuls reuse the stationary weights
                for kt in range(KT):
                    for sub in range(w // NCHUNK):
                        ns = slice(base + sub * NCHUNK, base + (sub + 1) * NCHUNK)
                        ps = slice(sub * NCHUNK, (sub + 1) * NCHUNK)
                        nc.tensor.matmul(L[:, ps], mkm_sb[:, kt, ms],
                                         x_sb[:, kt, ns],
                                         start=(kt == 0), stop=(kt == KT - 1))
                lsb = lpool.tile([128, w], bf16, tag="lsb")
                nc.vector.tensor_scalar_mul(lsb, L, SCALE)
                # exp reads the SBUF logits copy so the PSUM slot frees after
                # a single reader (DVE), keeping the PE accumulator pool fluid
                nc.scalar.activation(expl[:, mt, base:base + w], lsb,
                                     mybir.ActivationFunctionType.Exp)
                nc.sync.dma_start(out=lg_d[ms, base:base + w], in_=lsb)

        def stage2(nt0, ntiles, obs):
            nt = nt0
            for ob in obs:
                osb = opool.tile([128, ob, K], bf16, tag="osb")
                for t in range(ob):
                    ts_ = slice(nt * 128, (nt + 1) * 128)
                    O = psumO.tile([128, K + 1], f32, tag="O")
                    for mt in range(MT):
                        nc.tensor.matmul(O, expl[:, mt, ts_], mt1_sb[:, mt, :],
                                         start=(mt == 0), stop=(mt == MT - 1))
                    r = rpool.tile([128, 1], f32, tag="rcp")
                    nc.vector.reciprocal(r, O[:, K:K + 1])
                    # alternate the normalize between ACT and DVE to balance
                    if t % 2 == 0:
                        nc.scalar.mul(osb[:, t, :], O[:, 0:K], r)
                    else:
                        nc.vector.tensor_scalar_mul(osb[:, t, :], O[:, 0:K], r)
                    nt += 1
                nc.sync.dma_start(out=ot_re[:, nt - ob:nt, :], in_=osb)

        # ~2.5us of dummy matmuls so the PE clock gate is warm (and the
        # pipeline primed) by the time the first x bytes land
        Lw = psumL.tile([128, 512], f32, tag="L")
        for _ in range(22):
            nc.tensor.matmul(Lw, warm[:, 0:128], warm, start=True, stop=True)

        # stage 2 of block cc is emitted mid-way through stage 1 of block
        # cc+1, by which point all of block cc's exps have settled
        ntiles = LW // 128

        def make_mid(cc):
            obs = [8] if cc < NB - 1 else [4, 2, 2]
            return lambda: stage2(cc * ntiles, ntiles, obs)

        # first block in 512-wide granules to track the x stream
        stage1(0, NCHUNK)
        stage1(NCHUNK, NCHUNK)
        for cc in range(1, NB):
            stage1(cc * LW, LW, mid=make_mid(cc - 1))
        make_mid(NB - 1)()

    nc.compile()
    return nc


def _get_nc():
    if "nc" not in _cache:
        _cache["nc"] = _build()
    return _cache["nc"]


def _run(x, mem, trace=False, **kwargs):
    import ml_dtypes
    from concourse.bass_utils import run_bass_kernel_spmd

    nc = _get_nc()
    x = np.asarray(x, dtype=np.float32).reshape(B, K, N).astype(ml_dtypes.bfloat16)
    mem0 = np.asarray(mem, dtype=np.float32)[0]                    # (K, M)
    mkm = mem0.astype(ml_dtypes.bfloat16)
    mt1 = np.concatenate(
        [mem0.T, np.ones((M, 1), np.float32)], axis=1
    ).astype(ml_dtypes.bfloat16)                                   # (M, K+1)
    in_maps = [
        {"x": np.ascontiguousarray(x[b]), "mem_km": mkm, "mem_t1": mt1}
        for b in range(B)
    ]
    return run_bass_kernel_spmd(nc, in_maps, core_ids=list(range(NCORES)),
                                trace=trace, **kwargs)


def kernel(x, mask, mem):
    res = _run(x, mem, trace=False)
    logits = np.stack(
        [res.results[b]["logits"].astype(np.float32) for b in range(B)]
    )
    out = np.stack(
        [res.results[b]["outT"].astype(np.float32).T.reshape(K, HH, WW)
         for b in range(B)]
    )
    return out, logits


# revision 52
# speedup vs baseline: 1.0459x; 1.0230x over previous
"""Trainium2 Bass kernel for nn_AttentionMemoryBank.

Math (forward; mask/stop_gradient is identity in forward):
    xf     = x.reshape(B, K, N)                 # B=8, K=256, N=4096
    logits = einsum('km,bkn->bmn', mem[0], xf) / sqrt(K)   # (B, M=1024, N)
    attn   = softmax(logits, axis=1)            # over M
    out    = einsum('km,bmn->bkn', mem[0], attn).reshape(B, K, 64, 64)
    return (out, logits)

Distribution: data-parallel over batch — one batch element per NeuronCore
(8 cores), memory bank replicated.

Per-core kernel (bf16 matmuls, f32 PSUM accumulation; exp without
max-subtraction — logits ~ N(0,1) so exp is numerically safe). The N axis
is processed in 1024-wide blocks; per block:
  stage 1 (per 128-row mt tile): L (128m x 1024) = memKM.T @ x   (PE)
           lsb  = L * 1/16  bf16                                 (DVE) -> HBM
           expl = exp(lsb)  bf16                                 (ACT)
  stage 2 (per 128-wide n-tile, lagged half a block so exps settle):
           O (128n x 257) = sum_mt expl_mt[:, nt].T @ [memT | 1] (PE)
             -> O[:, :256] = unnormalized out.T, O[:, 256] = softmax denom
           outT = O[:, :256] * reciprocal(O[:, 256])  bf16       (DVE/ACT) -> HBM
Outputs are stored bf16 on device and upcast on host; the host also
transposes per-core outT (4096,256) -> (256,64,64) while stacking.
"""

import sys

if "/opt/trn_rl_repo" not in sys.path:
    sys.path.insert(0, "/opt/trn_rl_repo")

import numpy as np

B = 8
K = 256
HH = 64
WW = 64
N = HH * WW          # 4096
M = 1024             # mem bank size
NCORES = 8
NCHUNK = 512         # n-columns per stage-1 matmul (PSUM bank = 512 f32)
SCALE = 1.0 / 16.0   # 1/sqrt(K)

KT = K // 128        # 2 contraction tiles for stage 1
MT = M // 128        # 8 logits partition tiles / stage-2 contraction tiles
NCH = N // NCHUNK    # stage-1 chunks per row
NT = N // 128        # stage-2 n-tiles
OB = 4               # outT n-tiles batched per DMA

_cache = {}


def _build():
    from concourse import bacc
    import concourse.tile as tile
    from concourse import mybir
    from contextlib import ExitStack

    f32 = mybir.dt.float32
    bf16 = mybir.dt.bfloat16

    nc = bacc.Bacc("TRN2", target_bir_lowering=False, debug=False,
                   num_devices=NCORES)

    x_d = nc.dram_tensor("x", [K, N], bf16, kind="ExternalInput").ap()
    mkm_d = nc.dram_tensor("mem_km", [K, M], bf16, kind="ExternalInput").ap()
    mt1_d = nc.dram_tensor("mem_t1", [M, K + 1], bf16, kind="ExternalInput").ap()
    # outputs are stored bf16 on device (halves write traffic; the host
    # upcasts) — quantization adds ~1e-3 rel err, small next to the bf16
    # matmul error and far inside tolerance
    lg_d = nc.dram_tensor("logits", [M, N], bf16, kind="ExternalOutput").ap()
    ot_d = nc.dram_tensor("outT", [N, K], bf16, kind="ExternalOutput").ap()

    with tile.TileContext(nc) as tc, ExitStack() as ctx:
        const = ctx.enter_context(tc.tile_pool(name="const", bufs=1))

        x_sb = const.tile([128, KT, N], bf16)
        mkm_sb = const.tile([128, KT, M], bf16)
        mt1_sb = const.tile([128, MT, K + 1], bf16)
        x_re = x_d.rearrange("(t p) n -> p t n", p=128)
        mkm_re = mkm_d.rearrange("(t p) m -> p t m", p=128)
        mt1_re = mt1_d.rearrange("(t p) c -> p t c", p=128)
        # HWDGE issue order is what the first matmuls wait on: tiny first
        # slice of the memory bank, first x block, rest of the bank, the
        # bulk of x, with mt1 landing before stage 2 needs it
        nc.sync.dma_start(out=mkm_sb[:, :, 0:128], in_=mkm_re[:, :, 0:128])
        nc.sync.dma_start(out=x_sb[:, :, 0:512], in_=x_re[:, :, 0:512])
        nc.sync.dma_start(out=mt1_sb[:, 0:2, :], in_=mt1_re[:, 0:2, :])
        nc.sync.dma_start(out=mkm_sb[:, :, 128:M], in_=mkm_re[:, :, 128:M])
        nc.sync.dma_start(out=x_sb[:, :, 512:1024], in_=x_re[:, :, 512:1024])
        nc.sync.dma_start(out=mt1_sb[:, 2:MT, :], in_=mt1_re[:, 2:MT, :])
        nc.sync.dma_start(out=x_sb[:, :, 1024:2048], in_=x_re[:, :, 1024:2048])
        nc.sync.dma_start(out=x_sb[:, :, 2048:3072], in_=x_re[:, :, 2048:3072])
        nc.sync.dma_start(out=x_sb[:, :, 3072:N], in_=x_re[:, :, 3072:N])

        # warm up the PE (HAM clock gate) with throwaway matmuls on zeros
        # while the inputs stream in — PE is idle here anyway
        warm_pool = ctx.enter_context(tc.tile_pool(name="warm", bufs=1))
        warm = warm_pool.tile([128, 512], bf16)
        nc.vector.memset(warm, 0.0)

        psumL = ctx.enter_context(tc.tile_pool(name="psumL", bufs=3, space="PSUM"))
        psumO = ctx.enter_context(tc.tile_pool(name="psumO", bufs=2, space="PSUM"))
        lpool = ctx.enter_context(tc.tile_pool(name="lsb", bufs=12))
        epool = ctx.enter_context(tc.tile_pool(name="expl", bufs=1))
        opool = ctx.enter_context(tc.tile_pool(name="osb", bufs=2))
        rpool = ctx.enter_context(tc.tile_pool(name="rcp", bufs=4))

        # full exp(logits) stays resident: 8 x (128, 4096) bf16 = 64KB/partition
        expl = epool.tile([128, MT, N], bf16)
        ot_re = ot_d.rearrange("(g p) k -> p g k", p=128)

        LW = 2 * NCHUNK           # PSUM L spans 2 banks -> halves DVE/ACT ops
        NB = N // LW              # interleave blocks

        def stage1(base, w, mid=None):
            for mt in range(MT):
                if mt == 4 and mid is not None:
                    mid()
                ms = slice(mt * 128, (mt + 1) * 128)
                L = psumL.tile([128, w], f32, tag="L")
                # kt-major so consecutive matmuls reuse the stationary weights
                for kt in range(KT):
                    for sub in range(w // NCHUNK):
                        ns = slice(base + sub * NCHUNK, base + (sub + 1) * NCHUNK)
                        ps = slice(sub * NCHUNK, (sub + 1) * NCHUNK)
                        nc.tensor.matmul(L[:, ps], mkm_sb[:, kt, ms],
                                         x_sb[:, kt, ns],
                                         start=(kt == 0), stop=(kt == KT - 1))
                lsb = lpool.tile([128, w], bf16, tag="lsb")
                nc.vector.tensor_scalar_mul(lsb, L, SCALE)
                # exp reads the SBUF logits copy so the PSUM slot frees after
                # a single reader (DVE), keeping the PE accumulator pool fluid
                nc.scalar.activation(expl[:, mt, base:base + w], lsb,
                                     mybir.ActivationFunctionType.Exp)
                nc.sync.dma_start(out=lg_d[ms, base:base + w], in_=lsb)

        def stage2(nt0, ntiles, obs):
            nt = nt0
            for ob in obs:
                osb = opool.tile([128, ob, K], bf16, tag="osb")
                for t in range(ob):
                    ts_ = slice(nt * 128, (nt + 1) * 128)
                    O = psumO.tile([128, K + 1], f32, tag="O")
                    for mt in range(MT):
                        nc.tensor.matmul(O, expl[:, mt, ts_], mt1_sb[:, mt, :],
                                         start=(mt == 0), stop=(mt == MT - 1))
                    r = rpool.tile([128, 1], f32, tag="rcp")
                    nc.vector.reciprocal(r, O[:, K:K + 1])
                    # alternate the normalize between ACT and DVE to balance
                    if t % 2 == 0:
                        nc.scalar.mul(osb[:, t, :], O[:, 0:K], r)
                    else:
                        nc.vector.tensor_scalar_mul(osb[:, t, :], O[:, 0:K], r)
                    nt += 1
                nc.sync.dma_start(out=ot_re[:, nt - ob:nt, :], in_=osb)

        # ~2.5us of dummy matmuls so the PE clock gate is warm (and the
        # pipeline primed) by the time the first x bytes land
        Lw = psumL.tile([128, 512], f32, tag="L")
        for _ in range(22):
            nc.tensor.matmul(Lw, warm[:, 0:128], warm, start=True, stop=True)

        # stage 2 of block cc is emitted mid-way through stage 1 of block
        # cc+1, by which point all of block cc's exps have settled
        ntiles = LW // 128

        def make_mid(cc):
            obs = [8] if cc < NB - 1 else [4, 2, 2]
            return lambda: stage2(cc * ntiles, ntiles, obs)

        # first block in 512-wide granules to track the x stream
        stage1(0, NCHUNK)
        stage1(NCHUNK, NCHUNK)
        for cc in range(1, NB):
            stage1(cc * LW, LW, mid=make_mid(cc - 1))
        make_mid(NB - 1)()

    nc.compile()
    return nc


def _get_nc():
    if "nc" not in _cache:
        _cache["nc"] = _build()
    return _cache["nc"]


def _run(x, mem, trace=False, **kwargs):
    import ml_dtypes
    from concourse.bass_utils import run_bass_kernel_spmd

    nc = _get_nc()
    x = np.asarray(x, dtype=np.float32).reshape(B, K, N).astype(ml_dtypes.bfloat16)
    mem0 = np.asarray(mem, dtype=np.float32)[0]                    # (K, M)
    mkm = mem0.astype(ml_dtypes.bfloat16)
    mt1 = np.concatenate(
        [mem0.T, np.ones((M, 1), np.float32)], axis=1
    ).astype(ml_dtypes.bfloat16)                                   # (M, K+1)
    in_maps = [
        {"x": np.ascontiguousarray(x[b]), "mem_km": mkm, "mem_t1": mt1}
        for b in range(B)
    ]
    return run_bass_kernel_spmd(nc, in_maps, core_ids=list(range(NCORES)),
                                trace=trace, **kwargs)


def kernel(x, mask, mem):
    res = _run(x, mem, trace=False)
    logits = np.stack(
        [res.results[b]["logits"].astype(np.float32) for b in range(B)]
    )
    out = np.stack(
        [res.results[b]["outT"].astype(np.float32).T.reshape(K, HH, WW)
         for b in range(B)]
    )
    return out, logits
